# revision 1
# baseline (speedup 1.0000x reference)
"""MoE routing kernel (nn_HEA_10462540333708) for 8 Trainium2 NeuronCores.

Reference computation (B=16384, T=2, D=1024, DE=512, S=4, P=4):
    x = stack([x0, x1], 1)                                # [B, T, D]
    gates = softmax(x @ W_gate + b_gate)                  # [B, T, S+P]
    share = relu(x @ W_share + b_share)                   # [B, T, S, DE]
    spcf  = relu(x @ W_spcf  + b_spcf)                    # [B, T, P, DE]
    out   = einsum('bte,btef->btf', gates, [share|spcf])  # [B, T, DE]

Strategy: data-parallel over B across 8 cores (2048 rows each); the
small expert/gate weights are replicated, so there are no collectives.
The host transposes X to [d, b] tiles and casts X/W to bf16; on-chip
matmuls accumulate fp32 in PSUM, softmax + the gated mixture run in
fp32 on the Vector/Scalar engines.
"""

import numpy as np
import ml_dtypes

B, T, D, DE, S, P = 16384, 2, 1024, 512, 4, 4
NCORES = 8
BC = B // NCORES          # rows per core
MT = BC // 128            # m-tiles per task per core
KT = D // 128             # contraction tiles
NE = S + P                # experts per task
WCOLS = (S + T * P) * DE  # unique expert output columns

BF16 = ml_dtypes.bfloat16

_cache: dict = {}


def _build_bass(has_ebias: bool, has_gbias: bool):
    import concourse.bacc as bacc
    import concourse.mybir as mybir
    import concourse.tile as tile

    f32 = mybir.dt.float32
    bf16 = mybir.dt.bfloat16
    AX = mybir.AxisListType.X
    AF = mybir.ActivationFunctionType
    ALU = mybir.AluOpType

    nc = bacc.Bacc("TRN2", target_bir_lowering=False, debug=False)

    xt = nc.dram_tensor("xt", [T, MT, 128, KT, 128], bf16, kind="ExternalInput").ap()
    w = nc.dram_tensor("w", [128, KT, WCOLS], bf16, kind="ExternalInput").ap()
    wg = nc.dram_tensor("wg", [128, KT, T * NE], bf16, kind="ExternalInput").ap()
    out = nc.dram_tensor("out", [T, MT, 128, DE], f32, kind="ExternalOutput").ap()
    if has_ebias:
        # [shared 4 | task0-specific 4 | task1-specific 4], host-broadcast over partitions
        be = nc.dram_tensor("be", [128, S + T * P, DE], f32, kind="ExternalInput").ap()
    if has_gbias:
        bg = nc.dram_tensor("bg", [128, T * NE], f32, kind="ExternalInput").ap()

    NH = min(4, MT)  # head m-tiles processed expert-major during the W load

    def blk(t, e):
        return e if e < S else S + t * P + (e - S)

    with tile.TileContext(nc) as tc:
        with (
            tc.tile_pool(name="wp", bufs=1) as wpool,
            tc.tile_pool(name="xh", bufs=NH) as xhpool,
            tc.tile_pool(name="xp", bufs=3) as xpool,
            tc.tile_pool(name="hps", bufs=6, space="PSUM") as hpool,
            tc.tile_pool(name="gps", bufs=2, space="PSUM") as gppool,
            tc.tile_pool(name="act", bufs=4) as apool,
            tc.tile_pool(name="accp", bufs=NH + 2) as accpool,
            tc.tile_pool(name="soft", bufs=2) as spool,
        ):
            bet = bgt = None
            wgt = wpool.tile([128, KT, T * NE], bf16, tag="wg")
            nc.sync.dma_start(out=wgt[:], in_=wg[:, :, :])
            if has_ebias:
                bet = wpool.tile([128, S + T * P, DE], f32, tag="be")
                nc.sync.dma_start(out=bet[:], in_=be[:, :, :])
            if has_gbias:
                bgt = wpool.tile([128, T * NE], f32, tag="bg")
                nc.sync.dma_start(out=bgt[:], in_=bg[:, :])
            # xt for the head m-tiles lands before the expert weights
            xhead = []
            for m in range(NH):
                xts = xhpool.tile([128, KT, 128], bf16, tag=f"xh{m}")
                nc.sync.dma_start(out=xts[:], in_=xt[0, m])
                xhead.append(xts)
            # expert weights, one 512-col block per DMA, in the order the
            # head phase consumes them (task-0 blocks first)
            wblk = [None] * (S + T * P)
            for j in [blk(0, e) for e in range(NE)] + [blk(1, e) for e in range(S, NE)]:
                wt = wpool.tile([128, KT, DE], bf16, tag=f"w{j}")
                nc.sync.dma_start(out=wt[:], in_=w[:, :, j * DE : (j + 1) * DE])
                wblk[j] = wt

            def gates(t, xts, gtag="g"):
                gps = gppool.tile([128, NE], f32, tag="gp")
                for k in range(KT):
                    nc.tensor.matmul(
                        gps[:],
                        lhsT=xts[:, k, :],
                        rhs=wgt[:, k, t * NE : (t + 1) * NE],
                        start=(k == 0),
                        stop=(k == KT - 1),
                    )
                glog = gps
                if has_gbias:
                    glog = spool.tile([128, NE], f32, tag="glog")
                    nc.vector.tensor_tensor(
                        glog[:], gps[:], bgt[:, t * NE : (t + 1) * NE], op=ALU.add
                    )
                negmax = spool.tile([128, 1], f32, tag="negmax")
                nc.vector.tensor_reduce(
                    negmax[:], glog[:], axis=AX, op=ALU.max, negate=True
                )
                expg = spool.tile([128, NE], f32, tag="expg")
                nc.scalar.activation(expg[:], glog[:], AF.Exp, bias=negmax[:])
                ssum = spool.tile([128, 1], f32, tag="ssum")
                nc.vector.tensor_reduce(ssum[:], expg[:], axis=AX, op=ALU.add)
                rinv = spool.tile([128, 1], f32, tag="rinv")
                nc.vector.reciprocal(rinv[:], ssum[:])
                gsb = spool.tile([128, NE], f32, tag=gtag)
                nc.vector.tensor_scalar_mul(gsb[:], expg[:], rinv[:])
                return gsb

            def expert(t, e, xts, gsb, acc):
                hp = hpool.tile([128, DE], f32, tag="h")
                wt = wblk[blk(t, e)]
                for k in range(KT):
                    nc.tensor.matmul(
                        hp[:],
                        lhsT=xts[:, k, :],
                        rhs=wt[:, k, :],
                        start=(k == 0),
                        stop=(k == KT - 1),
                    )
                a = apool.tile([128, DE], f32, tag="a")
                if has_ebias:
                    tmp = apool.tile([128, DE], f32, tag="tmp")
                    nc.vector.tensor_tensor(
                        tmp[:], hp[:], bet[:, blk(t, e), :], op=ALU.add
                    )
                    nc.scalar.activation(a[:], tmp[:], AF.Relu)
                else:
                    nc.scalar.activation(a[:], hp[:], AF.Relu)
                if e == 0:
                    nc.vector.tensor_scalar_mul(acc[:], a[:], gsb[:, 0:1])
                else:
                    nc.vector.scalar_tensor_tensor(
                        acc[:], a[:], gsb[:, e : e + 1], acc[:],
                        op0=ALU.mult, op1=ALU.add,
                    )

            # head phase: task 0, m-tiles 0..NH-1, expert-major so PE
            # follows the arriving weight blocks instead of idling
            gh = [gates(0, xhead[m], gtag=f"gh{m}") for m in range(NH)]
            acch = []
            for m in range(NH):
                acc_h = accpool.tile([128, DE], f32, tag="acc")
                acch.append(acc_h)
            for e in range(NE):
                for m in range(NH):
                    expert(0, e, xhead[m], gh[m], acch[m])
            for m in range(NH):
                nc.gpsimd.dma_start(out=out[0, m], in_=acch[m][:])

            # steady state: m-major
            for t in range(T):
                for m in range(NH if t == 0 else 0, MT):
                    xts = xpool.tile([128, KT, 128], bf16, tag="x")
                    nc.sync.dma_start(out=xts[:], in_=xt[t, m])
                    gsb = gates(t, xts)
                    acc = accpool.tile([128, DE], f32, tag="acc")
                    for e in range(NE):
                        expert(t, e, xts, gsb, acc)
                    nc.gpsimd.dma_start(out=out[t, m], in_=acc[:])
    nc.compile()
    return nc


def _prep_weights(W_share, W_spcf, W_gate):
    # W_cat: [D, WCOLS] = [shared 4*DE | task0 P*DE | task1 P*DE]
    parts = [np.transpose(W_share, (1, 0, 2)).reshape(D, S * DE)]
    for t in range(T):
        parts.append(np.transpose(W_spcf[t], (1, 0, 2)).reshape(D, P * DE))
    W_cat = np.concatenate(parts, axis=1)
    w_host = np.ascontiguousarray(
        W_cat.reshape(KT, 128, WCOLS).transpose(1, 0, 2)
    ).astype(BF16)
    Wg_cat = np.transpose(W_gate, (1, 0, 2)).reshape(D, T * NE)
    wg_host = np.ascontiguousarray(
        Wg_cat.reshape(KT, 128, T * NE).transpose(1, 0, 2)
    ).astype(BF16)
    return w_host, wg_host


def _prep_x(x, core):
    # [BC, D] -> [MT, 128p(d%128), KT, 128b] bf16, contiguous per partition
    xc = x[core * BC : (core + 1) * BC]
    return np.ascontiguousarray(
        xc.reshape(MT, 128, KT, 128).transpose(0, 3, 2, 1)
    ).astype(BF16)


def kernel(x0, x1, W_share, b_share, W_spcf, b_spcf, W_gate, b_gate):
    from concourse.bass_utils import run_bass_kernel_spmd

    has_ebias = bool(np.any(b_share)) or bool(np.any(b_spcf))
    has_gbias = bool(np.any(b_gate))

    key = (has_ebias, has_gbias)
    if key not in _cache:
        _cache[key] = _build_bass(has_ebias, has_gbias)
    nc = _cache[key]

    w_host, wg_host = _prep_weights(
        np.asarray(W_share, np.float32),
        np.asarray(W_spcf, np.float32),
        np.asarray(W_gate, np.float32),
    )
    xs = [np.asarray(x0, np.float32), np.asarray(x1, np.float32)]

    in_maps = []
    for c in range(NCORES):
        m = {
            "xt": np.ascontiguousarray(
                np.stack([_prep_x(xs[t], c) for t in range(T)], axis=0)
            ),
            "w": w_host,
            "wg": wg_host,
        }
        if has_ebias:
            becat = np.concatenate(
                [
                    np.asarray(b_share, np.float32).reshape(S, DE),
                    np.asarray(b_spcf, np.float32).reshape(T * P, DE),
                ],
                axis=0,
            )
            m["be"] = np.ascontiguousarray(
                np.broadcast_to(becat[None], (128, S + T * P, DE))
            )
        if has_gbias:
            bgcat = np.asarray(b_gate, np.float32).reshape(T * NE)
            m["bg"] = np.ascontiguousarray(np.broadcast_to(bgcat[None], (128, T * NE)))
        in_maps.append(m)

    res = run_bass_kernel_spmd(nc, in_maps, core_ids=list(range(NCORES)))
    global _last_results
    _last_results = res

    out = np.empty((B, T, DE), np.float32)
    for c in range(NCORES):
        oc = res.results[c]["out"]  # [T, MT, 128, DE]
        out[c * BC : (c + 1) * BC] = oc.transpose(1, 2, 0, 3).reshape(BC, T, DE)
    return out

